# revision 18
# baseline (speedup 1.0000x reference)
"""NTM addressing head (nn_HeadBase) Trainium2 Bass kernel.

Full-input contract: kernel(**inputs) takes the unsharded [256, ...] arrays,
shards batch-dim across 8 NeuronCores (pure data parallel), runs one SPMD Bass
program per core, and gathers the full [256, 4096] output.

Per-core layout (B=32 batches, N=4096, M=64):
  memory[b] is streamed as [128, CB*2048] SBUF tiles (CB=2 batches/chunk,
  4-deep buffering) with n = p*32 + r (partition p, free = (b, r, m)); 8 KB
  contiguous per partition per batch.

  Phase A per chunk: the mem*k multiply is split GpSimd (1.5 batches) / DVE
  (0.5 batch), both writing an fp16 product tile; ACT squares mem into an
  fp16 tile.  The m=64 reductions run as fp16 tensor-tensor halving trees
  (64->32->16->8, 2x DVE rate) finished by a fp32-out native reduce (8->1).
  beta/||k|| is pre-folded into the broadcast k, so dot comes out pre-scaled.
  Small-input DMAs are issued from otherwise-idle engine sequencers so the
  Sync engine can start streaming memory chunks immediately.

  Phase B (all batches fused as [128, 1024] f32 tiles): a = dot * ssq^-0.5
  (tensor_scalar pow), softmax (no max-subtract: |a|<1), gated interpolation,
  3-tap circular shift via shifted APs; the +-1 partition carries go through
  two 128x128 circular-shift matmuls on the (otherwise idle) TensorEngine.
  w**gamma via tensor_tensor pow, final normalize.  Per-batch scalars are
  broadcast to [128, B] via K=1 ones-matmuls; PSUM evacuation on DVE.
"""

import numpy as np

B_FULL, N, M = 256, 4096, 64
NCORES = 8
B = B_FULL // NCORES   # 32 batches per core
P = 128                # SBUF partitions
R = N // P             # 32 rows per partition; n = p*R + r

_NC_CACHE = {}


def _build_body(nc, out_ap, ins):
    """Emit the kernel IR. ins: dict name->AP of DRAM inputs, out_ap: DRAM out."""
    from contextlib import ExitStack

    import concourse.bass as bass
    import concourse.tile as tile
    from concourse import mybir

    f32 = mybir.dt.float32
    f16 = mybir.dt.float16
    Alu = mybir.AluOpType
    Act = mybir.ActivationFunctionType
    Ax = mybir.AxisListType
    AP = bass.AP

    mem_ap = ins["memory"]   # [B, N, M]
    k_ap = ins["k"]          # [B, M]
    beta_ap = ins["beta"]    # [B, 1]
    pw_ap = ins["prev_w"]    # [B, N]
    g_ap = ins["g"]          # [B, 1]
    s_ap = ins["s"]          # [B, 3]
    gam_ap = ins["gamma"]    # [B, 1]

    def bcast_inner(ap2d, n):
        # [P, C] -> [P, C, n] with 0-stride inner dim
        return AP(ap2d.tensor, ap2d.offset, list(ap2d.ap) + [[0, n]])

    def row1(ap1d):
        # prepend a unit partition dim to a 1-d AP
        return AP(ap1d.tensor, ap1d.offset, [[0, 1]] + list(ap1d.ap))

    with tile.TileContext(nc) as tc, ExitStack() as ctx:
        singles = ctx.enter_context(tc.tile_pool(name="singles", bufs=1))
        mem_pool = ctx.enter_context(tc.tile_pool(name="mem", bufs=4))
        pr_pool = ctx.enter_context(tc.tile_pool(name="pr", bufs=2))
        sq_pool = ctx.enter_context(tc.tile_pool(name="sq", bufs=2))
        h_pool = ctx.enter_context(tc.tile_pool(name="h", bufs=2))
        big = ctx.enter_context(tc.tile_pool(name="big", bufs=1))
        ps = ctx.enter_context(tc.tile_pool(name="ps", bufs=2, space="PSUM"))
        ps_big = ctx.enter_context(tc.tile_pool(name="psbig", bufs=1, space="PSUM"))

        # ---- setup: constants, per-batch scalar rows on partition 0 ----
        # Small-input DMAs issued from idle engine sequencers (vector/scalar/
        # gpsimd/tensor) so the Sync engine's in-order queue is free to start
        # generating the big memory-chunk descriptors immediately.
        ones_col = singles.tile([P, 1], f32, tag="ones_col")
        nc.vector.memset(ones_col, 1.0)
        ones_row = singles.tile([1, P], f32, tag="ones_row")
        nc.vector.memset(ones_row, 1.0)

        k_row = singles.tile([1, B * M], f32, tag="k_row")
        nc.scalar.dma_start(out=k_row, in_=row1(k_ap.rearrange("b m -> (b m)")))
        b_row = singles.tile([1, B], f32, tag="b_row")
        nc.gpsimd.dma_start(out=b_row, in_=row1(beta_ap.rearrange("b one -> (b one)")))
        g_row = singles.tile([1, B], f32, tag="g_row")
        nc.gpsimd.dma_start(out=g_row, in_=row1(g_ap.rearrange("b one -> (b one)")))
        gm_row = singles.tile([1, B], f32, tag="gm_row")
        nc.gpsimd.dma_start(out=gm_row, in_=row1(gam_ap.rearrange("b one -> (b one)")))
        s_row = singles.tile([1, 3 * B], f32, tag="s_row")
        nc.scalar.dma_start(out=s_row, in_=row1(s_ap.rearrange("b i -> (b i)")))
        # prev_w big tile [P, B*R] in one permuted-AP DMA (128B inner runs)
        pw = big.tile([P, B * R], f32, tag="pw")
        nc.scalar.dma_start(
            out=pw.rearrange("p (b r) -> p b r", r=R),
            in_=pw_ap.rearrange("b (p r) -> p b r", r=R),
        )
        # s_i as [1, B] strided views (stride 3)
        s_perm = s_row.rearrange("p (b i) -> p i b", i=3)
        s_v = [s_perm[:, i, :] for i in range(3)]

        # knorm; bk = beta / knorm; fold into k so dot comes out pre-scaled.
        # (ksq is a scratch reusing kscl_row's allocation; dead before
        # kscl_row is written)
        kscl_row = singles.tile([1, B * M], f32, tag="kscl_row")
        ksq_row = kscl_row
        nc.vector.tensor_mul(ksq_row, k_row, k_row)
        ks_row = singles.tile([1, B], f32, tag="ks_row")
        nc.vector.tensor_reduce(
            out=ks_row, in_=ksq_row.rearrange("p (b m) -> p b m", m=M),
            axis=Ax.X, op=Alu.add,
        )
        # bk = beta / sqrt(ks)
        kn_row = singles.tile([1, B], f32, tag="kn_row")
        nc.scalar.activation(out=kn_row, in_=ks_row, func=Act.Sqrt)
        rk_row = singles.tile([1, B], f32, tag="rk_row")
        nc.vector.reciprocal(out=rk_row, in_=kn_row)
        bk_row = singles.tile([1, B], f32, tag="bk_row")
        nc.vector.tensor_mul(bk_row, b_row, rk_row)
        # kscl[b, m] = k[b, m] * beta[b] / ||k_b||
        nc.vector.tensor_mul(
            kscl_row.rearrange("p (b m) -> p b m", m=M),
            k_row.rearrange("p (b m) -> p b m", m=M),
            bcast_inner(bk_row, M),
        )

        # k broadcast to all partitions: kb[p, b*M+m] = kscl[b, m].
        # Matmult can carry only ONE sync-wait; kscl_row is DVE-produced so
        # both matmul deps (ones_row memset + k data) ride the DVE semaphore.
        kb_psum = ps_big.tile([P, B * M], f32, tag="kb_psum")
        for j in range(0, B * M, 512):
            nc.tensor.matmul(
                kb_psum[:, j : j + 512], ones_row, kscl_row[:, j : j + 512],
                start=True, stop=True,
            )
        kb = singles.tile([P, B * M], f32, tag="kb")
        nc.scalar.copy(out=kb, in_=kb_psum)

        # omg = 1 - g
        omg_row = singles.tile([1, B], f32, tag="omg_row")
        nc.vector.tensor_scalar(
            out=omg_row, in0=g_row, scalar1=-1.0, scalar2=1.0,
            op0=Alu.mult, op1=Alu.add,
        )

        # broadcast round 1: [omg, s0, s1, s2, gamma] -> [P, 5*B]
        NSC = 5
        asm1 = singles.tile([1, NSC * B], f32, tag="asm1")
        for i, src in enumerate([omg_row, s_v[0], s_v[1], s_v[2], gm_row]):
            nc.vector.tensor_copy(asm1[:, i * B : (i + 1) * B], src)
        bc1_ps = ps.tile([P, NSC * B], f32, tag="mm")
        nc.tensor.matmul(bc1_ps, ones_row, asm1, start=True, stop=True)
        BC1 = singles.tile([P, NSC * B], f32, tag="BC1")
        nc.vector.tensor_copy(BC1, bc1_ps)
        OMG = BC1[:, 0 * B : 1 * B]
        S0 = BC1[:, 1 * B : 2 * B]
        S1 = BC1[:, 2 * B : 3 * B]
        S2 = BC1[:, 3 * B : 4 * B]
        GAM = BC1[:, 4 * B : 5 * B]

        # pwo = prev_w * (1 - g): off the critical path, during phase A
        nc.vector.tensor_mul(
            pw.rearrange("p (b r) -> p b r", r=R),
            pw.rearrange("p (b r) -> p b r", r=R),
            bcast_inner(OMG, R),
        )

        # circular +-1 partition shift matrices for the conv carries:
        # SD[p, q] = 1 iff q == (p+1) mod P ; SU[p, q] = 1 iff q == (p-1) mod P
        ones_sq = singles.tile([P, P], f32, tag="ones_sq")
        nc.vector.memset(ones_sq, 1.0)
        sd_t = singles.tile([P, P], f32, tag="sd_t")
        nc.gpsimd.affine_select(
            out=sd_t, in_=ones_sq, pattern=[[1, P]], compare_op=Alu.is_equal,
            fill=0.0, base=-1, channel_multiplier=-1,
        )
        SD = singles.tile([P, P], f32, tag="SD")
        # wrap cell (127, 0): predicate q-p+127 != 0 everywhere except there
        nc.gpsimd.affine_select(
            out=SD, in_=sd_t, pattern=[[1, P]], compare_op=Alu.not_equal,
            fill=1.0, base=P - 1, channel_multiplier=-1,
        )
        su_t = singles.tile([P, P], f32, tag="su_t")
        nc.gpsimd.affine_select(
            out=su_t, in_=ones_sq, pattern=[[1, P]], compare_op=Alu.is_equal,
            fill=0.0, base=1, channel_multiplier=-1,
        )
        SU = singles.tile([P, P], f32, tag="SU")
        # wrap cell (0, 127): predicate q-p-127 != 0 everywhere except there
        nc.gpsimd.affine_select(
            out=SU, in_=su_t, pattern=[[1, P]], compare_op=Alu.not_equal,
            fill=1.0, base=-(P - 1), channel_multiplier=-1,
        )

        # ---- phase A: stream memory in CB-batch chunks ----
        CB = 2          # batches per chunk
        NCH = B // CB   # 16 chunks
        # GpSimd multiplies batch 0 + first GR rows of batch 1; DVE the rest.
        GR = 16
        dot = big.tile([P, B * R], f32, tag="dot")
        ssq = big.tile([P, B * R], f32, tag="ssq")
        for c in range(NCH):
            b0 = c * CB
            mt = mem_pool.tile([P, CB * R * M], f32, tag="mt")
            nc.sync.dma_start(
                out=mt.rearrange("p (b f) -> p b f", b=CB),
                in_=mem_ap[b0 : b0 + CB].rearrange(
                    "b (p r) m -> p b (r m)", p=P
                ),
            )
            mt4 = mt.rearrange("p (b r m) -> p b r m", b=CB, m=M)
            pr = pr_pool.tile([P, CB * R * M], f16, tag="pr")
            pr4 = pr.rearrange("p (b r m) -> p b r m", b=CB, m=M)
            kbc = kb[:, b0 * M : (b0 + CB) * M]  # [P, CB*M]
            # GpSimd: batch 0 full + batch 1 rows [0, GR)
            kb_b0 = AP(kbc.tensor, kbc.offset, [kbc.ap[0], [0, R], [1, M]])
            nc.gpsimd.tensor_tensor(
                out=pr4[:, 0], in0=mt4[:, 0], in1=kb_b0, op=Alu.mult
            )
            kb_b1 = AP(kbc.tensor, kbc.offset + M, [kbc.ap[0], [0, GR], [1, M]])
            nc.gpsimd.tensor_tensor(
                out=pr4[:, 1, 0:GR], in0=mt4[:, 1, 0:GR], in1=kb_b1, op=Alu.mult
            )
            # DVE: batch 1 rows [GR, R)
            kb_b1v = AP(
                kbc.tensor, kbc.offset + M, [kbc.ap[0], [0, R - GR], [1, M]]
            )
            nc.vector.tensor_tensor(
                out=pr4[:, 1, GR:R], in0=mt4[:, 1, GR:R], in1=kb_b1v, op=Alu.mult
            )
            # square on ACT, fp16 out
            sq = sq_pool.tile([P, CB * R * M], f16, tag="sq")
            nc.scalar.square(out=sq, in_=mt)

            # fp16 halving trees (2x DVE) + f32-out final reduce (8->1)
            G = CB * R  # 64 groups of 64
            for src, dst in ((pr, dot), (sq, ssq)):
                v64 = src.rearrange("p (g m) -> p g m", m=64)
                h1 = h_pool.tile([P, G * 32], f16, tag="h1")
                h1v = h1.rearrange("p (g m) -> p g m", m=32)
                nc.vector.tensor_add(out=h1v, in0=v64[:, :, 0:32], in1=v64[:, :, 32:64])
                h2 = h_pool.tile([P, G * 16], f16, tag="h2")
                h2v = h2.rearrange("p (g m) -> p g m", m=16)
                nc.vector.tensor_add(out=h2v, in0=h1v[:, :, 0:16], in1=h1v[:, :, 16:32])
                h3 = h_pool.tile([P, G * 8], f16, tag="h3")
                h3v = h3.rearrange("p (g m) -> p g m", m=8)
                nc.vector.tensor_add(out=h3v, in0=h2v[:, :, 0:8], in1=h2v[:, :, 8:16])
                nc.vector.tensor_reduce(
                    out=dst[:, b0 * R : (b0 + CB) * R].rearrange(
                        "p (b r) -> p b r", b=CB),
                    in_=h3v.rearrange("p (b r) m -> p (b r) m", b=CB),
                    axis=Ax.X, op=Alu.add,
                )
        # ---- phase B ----
        def v3(t):
            return t.rearrange("p (b r) -> p b r", r=R)

        # a = dot_scaled / sqrt(ssq)   (dot already has beta/||k|| folded in)
        # rstd via Newton-iterated approximate reciprocal on DVE: the exact
        # InstReciprocal costs 6.5us on HW for [128, 1024].
        nc.scalar.activation(out=ssq, in_=ssq, func=Act.Sqrt)
        # preload the Exp table while DVE runs the reciprocal, so the softmax
        # exp below doesn't pay the table switch on the critical path
        dummy = singles.tile([1, 1], f32, tag="dummy")
        nc.scalar.activation(out=dummy, in_=ks_row[:, 0:1], func=Act.Exp)
        ta = big.tile([P, B * R], f32, tag="ta")
        tb = big.tile([P, B * R], f32, tag="tb")
        nc.vector.reciprocal_approx_accurate(out=tb, in_=ssq, scratch=ta)
        nc.vector.tensor_mul(dot, dot, tb)

        # e = exp(a), in place
        nc.scalar.activation(out=dot, in_=dot, func=Act.Exp)
        e = dot

        # denom per batch; gd = g/denom
        cs = singles.tile([P, B], f32, tag="cs")
        nc.vector.tensor_reduce(out=cs, in_=v3(e), axis=Ax.X, op=Alu.add)
        den_ps = ps.tile([1, B], f32, tag="mm")
        nc.tensor.matmul(den_ps, ones_col, cs, start=True, stop=True)
        rden_row = singles.tile([1, B], f32, tag="rden_row")
        nc.vector.reciprocal(out=rden_row, in_=den_ps)
        gd_row = singles.tile([1, B], f32, tag="gd_row")
        nc.vector.tensor_mul(gd_row, rden_row, g_row)
        gd_ps = ps.tile([P, B], f32, tag="mm")
        nc.tensor.matmul(gd_ps, ones_row, gd_row, start=True, stop=True)
        GD = singles.tile([P, B], f32, tag="GD")
        nc.vector.tensor_copy(GD, gd_ps)

        # wg = e*gd + pwo   (in place into e)
        nc.vector.tensor_mul(v3(e), v3(e), bcast_inner(GD, R))
        nc.vector.tensor_add(out=e, in0=e, in1=pw)

        # circular 3-tap shift: ws[n] = s1*wg[n] + s0*wg[n-1] + s2*wg[n+1]
        ws = big.tile([P, B * R], f32, tag="ws")
        wg3, ws3, ta3, tb3 = v3(e), v3(ws), v3(ta), v3(tb)
        nc.vector.tensor_mul(ta3, wg3, bcast_inner(S0, R))
        nc.vector.tensor_mul(tb3, wg3, bcast_inner(S2, R))
        nc.vector.tensor_mul(ws3, wg3, bcast_inner(S1, R))
        # partition carries via circular-shift matmuls on the TensorEngine
        # (issued as soon as ta/tb are ready, overlapping the shifted adds):
        # dn[q, b] = ta[(q-1) mod P, b, R-1];  up[q, b] = tb[(q+1) mod P, b, 0]
        ta_col = AP(ta.tensor, ta.offset + (R - 1), [ta.ap[0], [R, B]])
        tb_col = AP(tb.tensor, tb.offset, [tb.ap[0], [R, B]])
        dn_ps = ps.tile([P, B], f32, tag="mm")
        nc.tensor.matmul(dn_ps, SD, ta_col, start=True, stop=True)
        up_ps = ps.tile([P, B], f32, tag="mm")
        nc.tensor.matmul(up_ps, SU, tb_col, start=True, stop=True)
        nc.vector.tensor_add(
            out=ws3[:, :, 1:R], in0=ws3[:, :, 1:R], in1=ta3[:, :, 0 : R - 1]
        )
        nc.vector.tensor_add(
            out=ws3[:, :, 0 : R - 1], in0=ws3[:, :, 0 : R - 1], in1=tb3[:, :, 1:R]
        )
        nc.vector.tensor_add(
            out=ws3[:, :, 0:1], in0=ws3[:, :, 0:1], in1=bcast_inner(dn_ps, 1)
        )
        nc.vector.tensor_add(
            out=ws3[:, :, R - 1 : R], in0=ws3[:, :, R - 1 : R],
            in1=bcast_inner(up_ps, 1),
        )

        # w_pow = ws ** gamma  (ALU pow on the GpSimd DSP; gamma per batch)
        nc.gpsimd.tensor_tensor(
            out=ws3, in0=ws3, in1=bcast_inner(GAM, R), op=Alu.pow
        )

        # normalize: out = w_pow / (sum + 1e-16)
        cs2 = singles.tile([P, B], f32, tag="cs2")
        nc.vector.tensor_reduce(out=cs2, in_=ws3, axis=Ax.X, op=Alu.add)
        d2_ps = ps.tile([1, B], f32, tag="mm")
        nc.tensor.matmul(d2_ps, ones_col, cs2, start=True, stop=True)
        d2_row = singles.tile([1, B], f32, tag="d2_row")
        nc.vector.tensor_scalar_add(out=d2_row, in0=d2_ps, scalar1=1e-16)
        rd2_row = singles.tile([1, B], f32, tag="rd2_row")
        nc.vector.reciprocal(out=rd2_row, in_=d2_row)
        rd2_ps = ps.tile([P, B], f32, tag="mm")
        nc.tensor.matmul(rd2_ps, ones_row, rd2_row, start=True, stop=True)
        RD2 = singles.tile([P, B], f32, tag="RD2")
        nc.vector.tensor_copy(RD2, rd2_ps)
        nc.vector.tensor_mul(ws3, ws3, bcast_inner(RD2, R))

        nc.sync.dma_start(
            out=out_ap.rearrange("b (p r) -> p b r", r=R),
            in_=ws.rearrange("p (b r) -> p b r", r=R),
        )


def _get_nc():
    if "nc" in _NC_CACHE:
        return _NC_CACHE["nc"]
    from concourse import bacc, mybir

    f32 = mybir.dt.float32
    nc = bacc.Bacc("TRN2", debug=False, num_devices=NCORES)
    ins = {
        "memory": nc.dram_tensor("memory", [B, N, M], f32, kind="ExternalInput").ap(),
        "k": nc.dram_tensor("k", [B, M], f32, kind="ExternalInput").ap(),
        "beta": nc.dram_tensor("beta", [B, 1], f32, kind="ExternalInput").ap(),
        "prev_w": nc.dram_tensor("prev_w", [B, N], f32, kind="ExternalInput").ap(),
        "g": nc.dram_tensor("g", [B, 1], f32, kind="ExternalInput").ap(),
        "s": nc.dram_tensor("s", [B, 3], f32, kind="ExternalInput").ap(),
        "gamma": nc.dram_tensor("gamma", [B, 1], f32, kind="ExternalInput").ap(),
    }
    out_ap = nc.dram_tensor("out", [B, N], f32, kind="ExternalOutput").ap()
    _build_body(nc, out_ap, ins)
    nc.finalize()
    _NC_CACHE["nc"] = nc
    return nc


def _shard_inputs(inputs):
    arrs = {
        name: np.ascontiguousarray(np.asarray(inputs[name], dtype=np.float32))
        for name in ("memory", "k", "beta", "prev_w", "g", "s", "gamma")
    }
    in_maps = []
    for c in range(NCORES):
        sl = slice(c * B, (c + 1) * B)
        in_maps.append({name: np.ascontiguousarray(a[sl]) for name, a in arrs.items()})
    return in_maps


def run(inputs, trace=False):
    from concourse.bass_utils import run_bass_kernel_spmd

    nc = _get_nc()
    in_maps = _shard_inputs(inputs)
    res = run_bass_kernel_spmd(
        nc, in_maps, core_ids=list(range(NCORES)), trace=trace,
        **({"trace_cores": [0]} if trace else {}),
    )
    out = np.concatenate([r["out"] for r in res.results], axis=0)
    return out, res


def kernel(**inputs):
    out, _ = run(inputs, trace=False)
    return out


# revision 19
# speedup vs baseline: 1.7452x; 1.7452x over previous
"""NTM addressing head (nn_HeadBase) Trainium2 Bass kernel.

Full-input contract: kernel(**inputs) takes the unsharded [256, ...] arrays,
shards batch-dim across 8 NeuronCores (pure data parallel), runs one SPMD Bass
program per core, and gathers the full [256, 4096] output.

Per-core layout (B=32 batches, N=4096, M=64):
  memory[b] is streamed as [128, CB*2048] SBUF tiles (CB=4 batches/chunk)
  with n = p*32 + r (partition p, free = (b, r, m)); 8 KB contiguous per
  partition per batch.

  Phase A per chunk: the mem*k multiply is split GpSimd (2.5 batches) / DVE
  (1.5 batches), both writing an fp16 product tile; ACT squares mem into an
  fp16 tile.  The m=64 reductions run as fp16 tensor-tensor halving trees
  (64->32->16->8, 2x DVE rate) finished by a fp32-out native reduce (8->1).
  Emission is software-pipelined: chunk c's mults are emitted before chunk
  c-1's trees so the in-order DVE queue reaches the mult (and frees the mem
  buffer for DMA c+2) without waiting behind tree work.
  Raw k is broadcast to all partitions immediately after its DMA; the
  beta/||k|| scale is applied as one extra phase-B multiply instead of
  delaying the broadcast.

  Phase B (all batches fused as [128, 1024] f32 tiles): a = beta/||k|| *
  dot / sqrt(ssq) (Newton-reciprocal on DVE; exact InstReciprocal costs
  6.5us), softmax (no max-subtract: |a|<1), gated interpolation, 3-tap
  circular shift via shifted APs; the +-1 partition carries go through two
  128x128 circular-shift matmuls on the idle TensorEngine.  pow via exp/ln
  with activation-table preloads hidden behind DVE work.  Per-batch scalars
  are broadcast to [128, B] via K=1 ones-matmuls; PSUM evacuation on DVE.
"""

import numpy as np

B_FULL, N, M = 256, 4096, 64
NCORES = 8
B = B_FULL // NCORES   # 32 batches per core
P = 128                # SBUF partitions
R = N // P             # 32 rows per partition; n = p*R + r

_NC_CACHE = {}


def _build_body(nc, out_ap, ins):
    """Emit the kernel IR. ins: dict name->AP of DRAM inputs, out_ap: DRAM out."""
    from contextlib import ExitStack

    import concourse.bass as bass
    import concourse.tile as tile
    from concourse import mybir

    f32 = mybir.dt.float32
    f16 = mybir.dt.float16
    Alu = mybir.AluOpType
    Act = mybir.ActivationFunctionType
    Ax = mybir.AxisListType
    AP = bass.AP

    mem_ap = ins["memory"]   # [B, N, M]
    k_ap = ins["k"]          # [B, M]
    beta_ap = ins["beta"]    # [B, 1]
    pw_ap = ins["prev_w"]    # [B, N]
    g_ap = ins["g"]          # [B, 1]
    s_ap = ins["s"]          # [B, 3]
    gam_ap = ins["gamma"]    # [B, 1]

    def bcast_inner(ap2d, n):
        # [P, C] -> [P, C, n] with 0-stride inner dim
        return AP(ap2d.tensor, ap2d.offset, list(ap2d.ap) + [[0, n]])

    def row1(ap1d):
        # prepend a unit partition dim to a 1-d AP
        return AP(ap1d.tensor, ap1d.offset, [[0, 1]] + list(ap1d.ap))

    with tile.TileContext(nc) as tc, ExitStack() as ctx:
        singles = ctx.enter_context(tc.tile_pool(name="singles", bufs=1))
        mem_pool = ctx.enter_context(tc.tile_pool(name="mem", bufs=2))
        pr_pool = ctx.enter_context(tc.tile_pool(name="pr", bufs=2))
        sq_pool = ctx.enter_context(tc.tile_pool(name="sq", bufs=2))
        h_pool = ctx.enter_context(tc.tile_pool(name="h", bufs=1))
        big = ctx.enter_context(tc.tile_pool(name="big", bufs=1))
        ps = ctx.enter_context(tc.tile_pool(name="ps", bufs=2, space="PSUM"))
        ps_big = ctx.enter_context(tc.tile_pool(name="psbig", bufs=1, space="PSUM"))

        # ---- setup ----
        # Small-input DMAs issued from idle engine sequencers (scalar/gpsimd)
        # so the Sync engine's in-order queue is free to start generating the
        # big memory-chunk descriptors immediately.
        ones_col = singles.tile([P, 1], f32, tag="ones_col")
        nc.vector.memset(ones_col, 1.0)
        ones_row = singles.tile([1, P], f32, tag="ones_row")
        nc.vector.memset(ones_row, 1.0)

        k_row = singles.tile([1, B * M], f32, tag="k_row")
        nc.scalar.dma_start(out=k_row, in_=row1(k_ap.rearrange("b m -> (b m)")))
        b_row = singles.tile([1, B], f32, tag="b_row")
        nc.gpsimd.dma_start(out=b_row, in_=row1(beta_ap.rearrange("b one -> (b one)")))
        g_row = singles.tile([1, B], f32, tag="g_row")
        nc.gpsimd.dma_start(out=g_row, in_=row1(g_ap.rearrange("b one -> (b one)")))
        gm_row = singles.tile([1, B], f32, tag="gm_row")
        nc.gpsimd.dma_start(out=gm_row, in_=row1(gam_ap.rearrange("b one -> (b one)")))
        s_row = singles.tile([1, 3 * B], f32, tag="s_row")
        nc.scalar.dma_start(out=s_row, in_=row1(s_ap.rearrange("b i -> (b i)")))
        # prev_w big tile [P, B*R] in one permuted-AP DMA (128B inner runs)
        pw = big.tile([P, B * R], f32, tag="pw")
        nc.scalar.dma_start(
            out=pw.rearrange("p (b r) -> p b r", r=R),
            in_=pw_ap.rearrange("b (p r) -> p b r", r=R),
        )
        # s_i as [1, B] strided views (stride 3)
        s_perm = s_row.rearrange("p (b i) -> p i b", i=3)
        s_v = [s_perm[:, i, :] for i in range(3)]

        # RAW k broadcast to all partitions ASAP: kb[p, b*M+m] = k[b, m].
        # Touch k on DVE first so the matmuls' deps ride the DVE semaphore.
        k_row2 = singles.tile([1, B * M], f32, tag="k_row2")
        nc.vector.tensor_copy(k_row2, k_row)
        kb_psum = ps_big.tile([P, B * M], f32, tag="kb_psum")
        for j in range(0, B * M, 512):
            nc.tensor.matmul(
                kb_psum[:, j : j + 512], ones_row, k_row2[:, j : j + 512],
                start=True, stop=True,
            )
        kb = singles.tile([P, B * M], f32, tag="kb")
        nc.scalar.copy(out=kb, in_=kb_psum)

        # bk = beta / ||k||  (computed in parallel with the broadcast; only
        # needed in phase B). ksq reuses k_row2's allocation (k_row2 is dead
        # once the matmuls consumed it -- enforced via Tile WAR tracking).
        ksq_row = singles.tile([1, B * M], f32, tag="ksq_row")
        nc.vector.tensor_mul(ksq_row, k_row, k_row)
        ks_row = singles.tile([1, B], f32, tag="ks_row")
        nc.vector.tensor_reduce(
            out=ks_row, in_=ksq_row.rearrange("p (b m) -> p b m", m=M),
            axis=Ax.X, op=Alu.add,
        )
        kn_row = singles.tile([1, B], f32, tag="kn_row")
        nc.scalar.activation(out=kn_row, in_=ks_row, func=Act.Sqrt)
        rk_row = singles.tile([1, B], f32, tag="rk_row")
        nc.vector.reciprocal(out=rk_row, in_=kn_row)
        bk_row = singles.tile([1, B], f32, tag="bk_row")
        nc.vector.tensor_mul(bk_row, b_row, rk_row)

        # omg = 1 - g
        omg_row = singles.tile([1, B], f32, tag="omg_row")
        nc.vector.tensor_scalar(
            out=omg_row, in0=g_row, scalar1=-1.0, scalar2=1.0,
            op0=Alu.mult, op1=Alu.add,
        )

        # broadcast round 1: [bk, omg, s0, s1, s2, gamma] -> [P, 6*B]
        NSC = 6
        asm1 = singles.tile([1, NSC * B], f32, tag="asm1")
        for i, src in enumerate([bk_row, omg_row, s_v[0], s_v[1], s_v[2], gm_row]):
            nc.vector.tensor_copy(asm1[:, i * B : (i + 1) * B], src)
        bc1_ps = ps.tile([P, NSC * B], f32, tag="mm")
        nc.tensor.matmul(bc1_ps, ones_row, asm1, start=True, stop=True)
        BC1 = singles.tile([P, NSC * B], f32, tag="BC1")
        nc.vector.tensor_copy(BC1, bc1_ps)
        BK = BC1[:, 0 * B : 1 * B]
        OMG = BC1[:, 1 * B : 2 * B]
        S0 = BC1[:, 2 * B : 3 * B]
        S1 = BC1[:, 3 * B : 4 * B]
        S2 = BC1[:, 4 * B : 5 * B]
        GAM = BC1[:, 5 * B : 6 * B]

        # pwo = prev_w * (1 - g): off the critical path, during phase A
        nc.vector.tensor_mul(
            pw.rearrange("p (b r) -> p b r", r=R),
            pw.rearrange("p (b r) -> p b r", r=R),
            bcast_inner(OMG, R),
        )

        # circular +-1 partition shift matrices for the conv carries:
        # SD[p, q] = 1 iff q == (p+1) mod P ; SU[p, q] = 1 iff q == (p-1) mod P
        ones_sq = singles.tile([P, P], f32, tag="ones_sq")
        nc.vector.memset(ones_sq, 1.0)
        sd_t = singles.tile([P, P], f32, tag="sd_t")
        nc.gpsimd.affine_select(
            out=sd_t, in_=ones_sq, pattern=[[1, P]], compare_op=Alu.is_equal,
            fill=0.0, base=-1, channel_multiplier=-1,
        )
        SD = singles.tile([P, P], f32, tag="SD")
        nc.gpsimd.affine_select(
            out=SD, in_=sd_t, pattern=[[1, P]], compare_op=Alu.not_equal,
            fill=1.0, base=P - 1, channel_multiplier=-1,
        )
        su_t = singles.tile([P, P], f32, tag="su_t")
        nc.gpsimd.affine_select(
            out=su_t, in_=ones_sq, pattern=[[1, P]], compare_op=Alu.is_equal,
            fill=0.0, base=1, channel_multiplier=-1,
        )
        SU = singles.tile([P, P], f32, tag="SU")
        nc.gpsimd.affine_select(
            out=SU, in_=su_t, pattern=[[1, P]], compare_op=Alu.not_equal,
            fill=1.0, base=-(P - 1), channel_multiplier=-1,
        )

        # ---- phase A: stream memory in CB-batch chunks, pipelined emission ----
        CB = 4          # batches per chunk
        NCH = B // CB   # 8 chunks
        GR = 16         # GpSimd takes batches [0, 2) + batch 2 rows [0, GR)
        dot = big.tile([P, B * R], f32, tag="dot")
        ssq = big.tile([P, B * R], f32, tag="ssq")

        def emit_stream(c):
            """DMA + mults + square for chunk c; returns (pr, sq) tiles."""
            b0 = c * CB
            mt = mem_pool.tile([P, CB * R * M], f32, tag="mt")
            nc.sync.dma_start(
                out=mt.rearrange("p (b f) -> p b f", b=CB),
                in_=mem_ap[b0 : b0 + CB].rearrange(
                    "b (p r) m -> p b (r m)", p=P
                ),
            )
            mt4 = mt.rearrange("p (b r m) -> p b r m", b=CB, m=M)
            pr = pr_pool.tile([P, CB * R * M], f16, tag="pr")
            pr4 = pr.rearrange("p (b r m) -> p b r m", b=CB, m=M)
            kbc = kb[:, b0 * M : (b0 + CB) * M]  # [P, CB*M]
            # GpSimd: batches 0-1 full + batch 2 rows [0, GR)
            kb_g0 = AP(kbc.tensor, kbc.offset, [kbc.ap[0], [M, 2], [0, R], [1, M]])
            nc.gpsimd.tensor_tensor(
                out=pr4[:, 0:2], in0=mt4[:, 0:2], in1=kb_g0, op=Alu.mult
            )
            kb_g1 = AP(kbc.tensor, kbc.offset + 2 * M, [kbc.ap[0], [0, GR], [1, M]])
            nc.gpsimd.tensor_tensor(
                out=pr4[:, 2, 0:GR], in0=mt4[:, 2, 0:GR], in1=kb_g1, op=Alu.mult
            )
            # DVE: batch 2 rows [GR, R) + batch 3
            kb_v0 = AP(
                kbc.tensor, kbc.offset + 2 * M, [kbc.ap[0], [0, R - GR], [1, M]]
            )
            nc.vector.tensor_tensor(
                out=pr4[:, 2, GR:R], in0=mt4[:, 2, GR:R], in1=kb_v0, op=Alu.mult
            )
            kb_v1 = AP(kbc.tensor, kbc.offset + 3 * M, [kbc.ap[0], [0, R], [1, M]])
            nc.vector.tensor_tensor(
                out=pr4[:, 3], in0=mt4[:, 3], in1=kb_v1, op=Alu.mult
            )
            # square on ACT, fp16 out
            sq = sq_pool.tile([P, CB * R * M], f16, tag="sq")
            nc.scalar.square(out=sq, in_=mt)
            return pr, sq

        def emit_trees(c, pr, sq):
            """fp16 halving trees (2x DVE) + f32-out final reduce for chunk c."""
            b0 = c * CB
            G = CB * R  # 128 groups of 64
            for src, dst in ((pr, dot), (sq, ssq)):
                v64 = src.rearrange("p (g m) -> p g m", m=64)
                h1 = h_pool.tile([P, G * 32], f16, tag="h1")
                h1v = h1.rearrange("p (g m) -> p g m", m=32)
                nc.vector.tensor_add(
                    out=h1v, in0=v64[:, :, 0:32], in1=v64[:, :, 32:64]
                )
                h2 = h_pool.tile([P, G * 16], f16, tag="h2")
                h2v = h2.rearrange("p (g m) -> p g m", m=16)
                nc.vector.tensor_add(
                    out=h2v, in0=h1v[:, :, 0:16], in1=h1v[:, :, 16:32]
                )
                h3 = h_pool.tile([P, G * 8], f16, tag="h3")
                h3v = h3.rearrange("p (g m) -> p g m", m=8)
                nc.vector.tensor_add(
                    out=h3v, in0=h2v[:, :, 0:8], in1=h2v[:, :, 8:16]
                )
                nc.vector.tensor_reduce(
                    out=dst[:, b0 * R : (b0 + CB) * R].rearrange(
                        "p (b r) -> p b r", b=CB),
                    in_=h3v.rearrange("p (b r) m -> p (b r) m", b=CB),
                    axis=Ax.X, op=Alu.add,
                )

        prev = emit_stream(0)
        for c in range(1, NCH):
            cur = emit_stream(c)
            emit_trees(c - 1, *prev)
            prev = cur
        # preload the Sqrt table while the last chunk computes
        dummy = singles.tile([1, 1], f32, tag="dummy")
        nc.scalar.activation(out=dummy, in_=ks_row[:, 0:1], func=Act.Sqrt)
        emit_trees(NCH - 1, *prev)

        # ---- phase B ----
        def v3(t):
            return t.rearrange("p (b r) -> p b r", r=R)

        # a = (beta/||k||) * dot / sqrt(ssq)
        nc.scalar.activation(out=ssq, in_=ssq, func=Act.Sqrt)
        # preload the Exp table while DVE runs the reciprocal
        nc.scalar.activation(out=dummy, in_=ks_row[:, 0:1], func=Act.Exp)
        ta = big.tile([P, B * R], f32, tag="ta")
        tb = big.tile([P, B * R], f32, tag="tb")
        nc.vector.reciprocal_approx_accurate(out=tb, in_=ssq, scratch=ta)
        nc.vector.tensor_mul(dot, dot, tb)
        nc.vector.tensor_mul(v3(dot), v3(dot), bcast_inner(BK, R))

        # e = exp(a), in place
        nc.scalar.activation(out=dot, in_=dot, func=Act.Exp)
        e = dot
        # preload the Ln table while DVE runs the softmax/gating chain
        nc.scalar.activation(out=dummy, in_=ks_row[:, 0:1], func=Act.Ln)

        # denom per batch; gd = g/denom
        cs = singles.tile([P, B], f32, tag="cs")
        nc.vector.tensor_reduce(out=cs, in_=v3(e), axis=Ax.X, op=Alu.add)
        den_ps = ps.tile([1, B], f32, tag="mm")
        nc.tensor.matmul(den_ps, ones_col, cs, start=True, stop=True)
        rden_row = singles.tile([1, B], f32, tag="rden_row")
        nc.vector.reciprocal(out=rden_row, in_=den_ps)
        gd_row = singles.tile([1, B], f32, tag="gd_row")
        nc.vector.tensor_mul(gd_row, rden_row, g_row)
        gd_ps = ps.tile([P, B], f32, tag="mm")
        nc.tensor.matmul(gd_ps, ones_row, gd_row, start=True, stop=True)
        GD = singles.tile([P, B], f32, tag="GD")
        nc.vector.tensor_copy(GD, gd_ps)

        # wg = e*gd + pwo   (in place into e)
        nc.vector.tensor_mul(v3(e), v3(e), bcast_inner(GD, R))
        nc.vector.tensor_add(out=e, in0=e, in1=pw)

        # circular 3-tap shift: ws[n] = s1*wg[n] + s0*wg[n-1] + s2*wg[n+1]
        ws = big.tile([P, B * R], f32, tag="ws")
        wg3, ws3, ta3, tb3 = v3(e), v3(ws), v3(ta), v3(tb)
        nc.vector.tensor_mul(ta3, wg3, bcast_inner(S0, R))
        nc.vector.tensor_mul(tb3, wg3, bcast_inner(S2, R))
        nc.vector.tensor_mul(ws3, wg3, bcast_inner(S1, R))
        # partition carries via circular-shift matmuls on the TensorEngine
        # (issued as soon as ta/tb are ready, overlapping the shifted adds):
        # dn[q, b] = ta[(q-1) mod P, b, R-1];  up[q, b] = tb[(q+1) mod P, b, 0]
        ta_col = AP(ta.tensor, ta.offset + (R - 1), [ta.ap[0], [R, B]])
        tb_col = AP(tb.tensor, tb.offset, [tb.ap[0], [R, B]])
        dn_ps = ps.tile([P, B], f32, tag="mm")
        nc.tensor.matmul(dn_ps, SD, ta_col, start=True, stop=True)
        up_ps = ps.tile([P, B], f32, tag="mm")
        nc.tensor.matmul(up_ps, SU, tb_col, start=True, stop=True)
        nc.vector.tensor_add(
            out=ws3[:, :, 1:R], in0=ws3[:, :, 1:R], in1=ta3[:, :, 0 : R - 1]
        )
        nc.vector.tensor_add(
            out=ws3[:, :, 0 : R - 1], in0=ws3[:, :, 0 : R - 1], in1=tb3[:, :, 1:R]
        )
        nc.vector.tensor_add(
            out=ws3[:, :, 0:1], in0=ws3[:, :, 0:1], in1=bcast_inner(dn_ps, 1)
        )
        nc.vector.tensor_add(
            out=ws3[:, :, R - 1 : R], in0=ws3[:, :, R - 1 : R],
            in1=bcast_inner(up_ps, 1),
        )

        # w_pow = ws ** gamma = exp(gamma * ln(ws))
        nc.scalar.activation(out=ws, in_=ws, func=Act.Ln)
        # preload the Exp table while DVE runs the gamma multiply
        nc.scalar.activation(out=dummy, in_=ks_row[:, 0:1], func=Act.Exp)
        nc.vector.tensor_mul(ws3, ws3, bcast_inner(GAM, R))
        nc.scalar.activation(out=ws, in_=ws, func=Act.Exp)

        # normalize: out = w_pow / (sum + 1e-16)
        cs2 = singles.tile([P, B], f32, tag="cs2")
        nc.vector.tensor_reduce(out=cs2, in_=ws3, axis=Ax.X, op=Alu.add)
        d2_ps = ps.tile([1, B], f32, tag="mm")
        nc.tensor.matmul(d2_ps, ones_col, cs2, start=True, stop=True)
        d2_row = singles.tile([1, B], f32, tag="d2_row")
        nc.vector.tensor_scalar_add(out=d2_row, in0=d2_ps, scalar1=1e-16)
        rd2_row = singles.tile([1, B], f32, tag="rd2_row")
        nc.vector.reciprocal(out=rd2_row, in_=d2_row)
        rd2_ps = ps.tile([P, B], f32, tag="mm")
        nc.tensor.matmul(rd2_ps, ones_row, rd2_row, start=True, stop=True)
        RD2 = singles.tile([P, B], f32, tag="RD2")
        nc.vector.tensor_copy(RD2, rd2_ps)
        nc.vector.tensor_mul(ws3, ws3, bcast_inner(RD2, R))

        nc.sync.dma_start(
            out=out_ap.rearrange("b (p r) -> p b r", r=R),
            in_=ws.rearrange("p (b r) -> p b r", r=R),
        )


def _get_nc():
    if "nc" in _NC_CACHE:
        return _NC_CACHE["nc"]
    from concourse import bacc, mybir

    f32 = mybir.dt.float32
    nc = bacc.Bacc("TRN2", debug=False, num_devices=NCORES)
    ins = {
        "memory": nc.dram_tensor("memory", [B, N, M], f32, kind="ExternalInput").ap(),
        "k": nc.dram_tensor("k", [B, M], f32, kind="ExternalInput").ap(),
        "beta": nc.dram_tensor("beta", [B, 1], f32, kind="ExternalInput").ap(),
        "prev_w": nc.dram_tensor("prev_w", [B, N], f32, kind="ExternalInput").ap(),
        "g": nc.dram_tensor("g", [B, 1], f32, kind="ExternalInput").ap(),
        "s": nc.dram_tensor("s", [B, 3], f32, kind="ExternalInput").ap(),
        "gamma": nc.dram_tensor("gamma", [B, 1], f32, kind="ExternalInput").ap(),
    }
    out_ap = nc.dram_tensor("out", [B, N], f32, kind="ExternalOutput").ap()
    _build_body(nc, out_ap, ins)
    nc.finalize()
    _NC_CACHE["nc"] = nc
    return nc


def _shard_inputs(inputs):
    arrs = {
        name: np.ascontiguousarray(np.asarray(inputs[name], dtype=np.float32))
        for name in ("memory", "k", "beta", "prev_w", "g", "s", "gamma")
    }
    in_maps = []
    for c in range(NCORES):
        sl = slice(c * B, (c + 1) * B)
        in_maps.append({name: np.ascontiguousarray(a[sl]) for name, a in arrs.items()})
    return in_maps


def run(inputs, trace=False):
    from concourse.bass_utils import run_bass_kernel_spmd

    nc = _get_nc()
    in_maps = _shard_inputs(inputs)
    res = run_bass_kernel_spmd(
        nc, in_maps, core_ids=list(range(NCORES)), trace=trace,
        **({"trace_cores": [0]} if trace else {}),
    )
    out = np.concatenate([r["out"] for r in res.results], axis=0)
    return out, res


def kernel(**inputs):
    out, _ = run(inputs, trace=False)
    return out
